# revision 20
# baseline (speedup 1.0000x reference)
import numpy as np
import ml_dtypes

import concourse.bacc as bacc
import concourse.mybir as mybir
import concourse.tile as tile
from concourse.bass_utils import run_bass_kernel_spmd

# Problem constants (hardcoded per harness contract)
B, H, W, C = 32, 32, 32, 128
NUM, D0, D1 = 10, 60, 16
JK = D0 * D1            # 960
OO = NUM * JK           # 9600
P = H * W               # 1024 contraction dim of the dense kernel
N_CORES = 8
B_LOC = B // N_CORES    # 4 batches per core
EPS = 1e-12

f32 = mybir.dt.float32
f32r = mybir.dt.float32r
bf16 = mybir.dt.bfloat16
AF = mybir.ActivationFunctionType
ALU = mybir.AluOpType


def build_nc():
    nc = bacc.Bacc("TRN2", debug=False)
    # host pre-laid-out inputs (see _prep_* below)
    u_d = nc.dram_tensor("u", (C, B_LOC, P), f32r, kind="ExternalInput").ap()
    wc_d = nc.dram_tensor("wc", (C, 4, C), f32r, kind="ExternalInput").ap()
    km_d = nc.dram_tensor("km", (NUM, 128, 8, JK), bf16, kind="ExternalInput").ap()
    out_d = nc.dram_tensor("out", (B_LOC, NUM, JK), f32, kind="ExternalOutput").ap()

    with tile.TileContext(nc) as tc:
        with tc.tile_pool(name="persist", bufs=1) as pers, \
             tc.tile_pool(name="kp", bufs=4) as kp:
            u_hat = pers.tile([128, B_LOC, OO], bf16)       # [n, b, o]
            uT = pers.tile([128, B_LOC, 8, 128], bf16)      # lhsT chunks [p, b, chunk, c]
            wct = pers.tile([128, 4, C], f32r)              # conv taps [ci, tap, co]
            ones = pers.tile([128, 128], bf16)
            crep0 = pers.tile([128, 128], bf16)             # uniform c = 0.1 (softmax of zeros)
            crep_all = pers.tile([128, B_LOC, NUM, 128], bf16)
            crep2m = pers.tile([128, B_LOC, NUM, NUM], bf16)  # masked cols, final combine
            c_all = pers.tile([128, B_LOC, NUM], f32)
            z_all = pers.tile([128, B_LOC, NUM], f32)
            ss_all = pers.tile([128, B_LOC, NUM], f32)
            alpha = pers.tile([128, B_LOC, NUM], f32)
            blog = pers.tile([128, B_LOC, NUM], f32)
            eexp = pers.tile([128, B_LOC, NUM], f32)
            nmax = pers.tile([128, B_LOC], f32)
            sume = pers.tile([128, B_LOC], f32)
            rsum = pers.tile([128, B_LOC], f32)
            xpadA = pers.tile([128, 33 * 33], f32r)
            xpadB = pers.tile([128, 33 * 33], f32r)
            xpads = [xpadA, xpadB]
            xin_all = pers.tile([128, B_LOC, P], f32r)

            # conv inputs stream on the gpsimd queue; dense-kernel capsule
            # blocks stream on the otherwise-idle sync queue
            nc.sync.dma_start(wct[:], wc_d)
            nc.sync.dma_start(xin_all[:], u_d)
            kt_a = kp.tile([128, 8, JK], bf16, tag="kt")
            kt_b = kp.tile([128, 8, JK], bf16, tag="kt")
            kts = [kt_a, kt_b]
            nc.gpsimd.dma_start(kts[0][:, 0:4], km_d[0, :, 0:4])
            nc.gpsimd.dma_start(kts[0][:, 4:8], km_d[0, :, 4:8])
            nc.scalar.dma_start(kts[1][:, 0:4], km_d[1, :, 0:4])
            nc.scalar.dma_start(kts[1][:, 4:8], km_d[1, :, 4:8])

            nc.vector.memset(ones[:], 1.0)
            nc.vector.memset(crep0[:], 0.1)
            nc.vector.memset(crep2m[:], 0.0)
            nc.vector.memset(xpads[0][:].bitcast(f32), 0.0)
            nc.vector.memset(xpads[1][:].bitcast(f32), 0.0)
            xpad_vs = [x[:].rearrange("p (h w) -> p h w", w=33) for x in xpads]

            # ---------- Phase 1: 2x2 SAME conv, per batch ----------
            # out[co, s=h*32+w] = sum_taps Wtap.T @ xpad[:, (h+dh)*33 + (w+dw)]
            with tc.tile_pool(name="psc", bufs=2, space="PSUM") as psc:
                for b in range(B_LOC):
                    xpad_v = xpad_vs[b % 2]
                    src = xin_all[:, b].rearrange("p (h w) -> p h w", w=32)
                    if b % 2 == 0:
                        nc.vector.tensor_copy(xpad_v[:, 0:32, 0:32], src)
                    else:
                        nc.scalar.copy(xpad_v[:, 0:32, 0:32], src)
                    for hh in range(2):
                        pc = psc.tile([128, 512], f32, tag="pc")
                        for ti, (dh, dw) in enumerate(((0, 0), (0, 1), (1, 0), (1, 1))):
                            rhs = xpad_v[:, hh * 16 + dh: hh * 16 + dh + 16, dw:dw + 32]
                            nc.tensor.matmul(pc[:], wct[:, ti, :], rhs,
                                             start=(ti == 0), stop=(ti == 3))
                        # raw-reshape gather: uT[t][pp, c] = conv[a, 8q+t, pp], c = 4a+q
                        pcv = pc[:].rearrange("p (a q t) -> p a q t", q=4, t=8)
                        for t in range(8):
                            src = pcv[:, :, :, t]
                            dst = uT[:, b, t, hh * 64:(hh + 1) * 64].rearrange(
                                "p (a q) -> p a q", q=4)
                            if t % 2 == 0:
                                nc.vector.tensor_copy(dst, src)
                            else:
                                nc.scalar.copy(dst, src)

            def softmax_b(b):
                # b_logits = z * rsqrt(max(ss, eps)); softmax over capsules -> c_all
                nc.vector.tensor_scalar_max(ss_all[:, b], ss_all[:, b], EPS)
                nc.scalar.activation(ss_all[:, b], ss_all[:, b], AF.Sqrt)
                nc.vector.reciprocal(alpha[:, b], ss_all[:, b])
                nc.vector.tensor_mul(blog[:, b], z_all[:, b], alpha[:, b])
                nc.vector.tensor_reduce(nmax[:, b:b + 1], blog[:, b],
                                        axis=mybir.AxisListType.X,
                                        op=ALU.max, negate=True)
                nc.scalar.activation(eexp[:, b], blog[:, b], AF.Exp,
                                     bias=nmax[:, b:b + 1],
                                     accum_out=sume[:, b:b + 1])
                nc.vector.reciprocal(rsum[:, b:b + 1], sume[:, b:b + 1])
                nc.vector.tensor_scalar_mul(c_all[:, b], eexp[:, b],
                                            rsum[:, b:b + 1])

            with tc.tile_pool(name="rt", bufs=3) as rt, \
                 tc.tile_pool(name="ps", bufs=4, space="PSUM") as ps:

                def zss_update(b, i, it1):
                    # o for (b, capsule i) -> PSUM broadcast; z/ss accumulations
                    o0 = i * JK
                    pbc = ps.tile([128, JK], f32, tag="pbc")
                    lhs = crep_all[:, b, i] if it1 else crep0[:]
                    nc.tensor.matmul(pbc[:, 0:512], lhs,
                                     u_hat[:, b, o0:o0 + 512],
                                     start=True, stop=True)
                    nc.tensor.matmul(pbc[:, 512:JK], lhs,
                                     u_hat[:, b, o0 + 512:o0 + JK],
                                     start=True, stop=True)
                    scr = rt.tile([128, JK], bf16, tag="scr")
                    nc.vector.scalar_tensor_tensor(
                        out=scr[:],
                        in0=u_hat[:, b, o0:o0 + JK],
                        scalar=1.0, in1=pbc[:],
                        op0=ALU.mult, op1=ALU.mult,
                        accum_out=z_all[:, b, i:i + 1])
                    scr2 = rt.tile([128, JK], bf16, tag="scr2")
                    nc.scalar.activation(
                        scr2[:], pbc[:], AF.Square,
                        accum_out=ss_all[:, b, i:i + 1])

                # ---- Phase 2: dense GEMM per capsule, iteration-0 routing woven in ----
                for cap in range(NUM):
                    if cap + 2 < NUM:
                        ktn = kp.tile([128, 8, JK], bf16, tag="kt")
                        kts.append(ktn)
                        nc.sync.dma_start(ktn[:, 0:4], km_d[cap + 2, :, 0:4])
                        nc.gpsimd.dma_start(ktn[:, 4:8], km_d[cap + 2, :, 4:8])
                    kt = kts[cap]
                    o0 = cap * JK
                    for b in range(B_LOC):
                        pm = ps.tile([128, JK], f32, tag="pbc")
                        for ch in range(8):
                            nc.tensor.matmul(pm[:, 0:512], uT[:, b, ch, :],
                                             kt[:, ch, 0:512],
                                             start=(ch == 0), stop=(ch == 7))
                        for ch in range(8):
                            nc.tensor.matmul(pm[:, 512:JK], uT[:, b, ch, :],
                                             kt[:, ch, 512:JK],
                                             start=(ch == 0), stop=(ch == 7))
                        dst = u_hat[:, b, o0:o0 + JK]
                        if b % 2 == 0:
                            nc.vector.tensor_copy(dst, pm[:])
                        else:
                            nc.scalar.copy(dst, pm[:])
                        # routing iteration 0 woven per batch (c uniform = 0.1)
                        zss_update(b, cap, False)
                        if cap == NUM - 1:
                            # all capsules of b are in flight -> logits+weights
                            softmax_b(b)
                            for i in range(NUM):
                                nc.gpsimd.tensor_scalar_mul(
                                    crep_all[:, b, i], ones[:],
                                    c_all[:, b, i:i + 1])

                # ---- Phase 3: routing iterations 1 (update) and 2 (final) ----
                def it1_group(b):
                    for i in range(NUM):
                        zss_update(b, i, True)
                    softmax_b(b)
                    for i in range(NUM):
                        nc.gpsimd.tensor_scalar_mul(
                            crep2m[:, b, i, i:i + 1], ones[:, 0:1],
                            c_all[:, b, i:i + 1])

                def it2_group(b):
                    # all 10 capsules' o accumulated into one [10, 960] tile via
                    # masked lhsT columns, then one copy + one DMA per batch
                    pf = ps.tile([128, JK], f32, tag="pbc")
                    for i in range(NUM):
                        nc.tensor.matmul(pf[0:NUM, 0:512], crep2m[:, b, i],
                                         u_hat[:, b, i * JK:i * JK + 512],
                                         start=(i == 0), stop=(i == NUM - 1))
                    for i in range(NUM):
                        nc.tensor.matmul(pf[0:NUM, 512:JK], crep2m[:, b, i],
                                         u_hat[:, b, i * JK + 512:(i + 1) * JK],
                                         start=(i == 0), stop=(i == NUM - 1))
                    ofin = rt.tile([NUM, JK], f32, tag="ofin")
                    if b % 2 == 0:
                        nc.vector.tensor_copy(ofin[:], pf[0:NUM, :])
                    else:
                        nc.scalar.copy(ofin[:], pf[0:NUM, :])
                    nc.sync.dma_start(out_d[b], ofin[:])

                for b in range(B_LOC):
                    it1_group(b)
                for b in range(B_LOC):
                    it2_group(b)
    nc.compile()
    return nc


_NC_CACHE = None


def _get_nc():
    global _NC_CACHE
    if _NC_CACHE is None:
        _NC_CACHE = build_nc()
    return _NC_CACHE


def _prep_km(km):
    # km[cap, p, c, col] = K[c*128 + p, cap*960 + col], contiguous per partition
    kt = km.reshape(8, 128, NUM, JK).transpose(2, 1, 0, 3)
    return np.ascontiguousarray(kt).astype(ml_dtypes.bfloat16)


def _prep_u(u):
    # NHWC -> per-core [C, B_LOC, H*W]: channel-major (no on-device
    # transposes) and contiguous per partition for the one-shot DMA
    v = u.transpose(0, 3, 1, 2).reshape(N_CORES, B_LOC, C, P).transpose(0, 2, 1, 3)
    return np.ascontiguousarray(v)


def _prep_wc(wc):
    # [dh, dw, ci, co] -> [ci, (dh dw), co]
    return np.ascontiguousarray(wc.transpose(2, 0, 1, 3).reshape(C, 4, C))


def kernel(u_vecs, W_conv, kernel):
    u_vecs = _prep_u(np.asarray(u_vecs, dtype=np.float32))
    wc = _prep_wc(np.asarray(W_conv, dtype=np.float32))
    km = _prep_km(np.asarray(kernel, dtype=np.float32))
    nc = _get_nc()
    in_maps = [
        {"u": u_vecs[ci], "wc": wc, "km": km}
        for ci in range(N_CORES)
    ]
    res = run_bass_kernel_spmd(nc, in_maps, core_ids=list(range(N_CORES)))
    out = np.concatenate([r["out"] for r in res.results], axis=0)
    return out.reshape(B, NUM, D0, D1).astype(np.float32)
